# revision 19
# baseline (speedup 1.0000x reference)
"""Trainium2 Bass kernel for the Kruskal (CP/Tucker) linear layer.

Math: the reference reconstructs W (4096x4096) from a rank-16 CP core and
Tucker factors, then computes y = x @ W.T + bias.  Because the 6D core is a
CP (Kruskal) tensor of rank 16, W itself is exactly rank 16:

    W = g_out @ g_in.T
    g_in[def, r]  = (f3@c3)[d,r] * (f4@c4)[e,r] * (f5@c5)[f,r]   (4096 x 16)
    g_out[abc, r] = (f0@c0)[a,r] * (f1@c1)[b,r] * (f2@c2)[c,r]   (4096 x 16)

so  y = (x @ g_in) @ g_out.T + bias.  The device kernel computes the two
x-dependent projections; the tiny factor-only products (g_in/g_out, ~100
KFLOP) are prepared on the host.

Sharding: data-parallel over the batch (4096 rows -> 8 cores x 512). No
collectives.  The host ships each core its x slice PRE-TRANSPOSED and cast
to bf16 (x^T slice, 4096 features x 512 batch).  Feature-major HBM layout
means stage 1 needs no on-device transpose at all: the contraction dim
lands on partitions straight off the DMA.  Per core:
  1. 8 HWDGE loads of x^T k-groups (128, 4, 512) bf16
  2. stage 1: 32 accumulating matmuls  t^T(16,512) += g_in_kt.T @ x^T_kt
     (N=512, one PSUM bank for the whole core's t^T)
  3. DVE copy t^T -> SBUF bf16 (rank rows + ones row for the bias)
  4. stage 2: 32 bf16 matmuls (4 batch tiles x 8 col tiles, N=512)
     y = [t,1] @ [g_out.T; bias]
  5. DVE/ACT copy PSUM->SBUF, DMA y fp32 out per batch tile
"""

import numpy as np
import ml_dtypes

N_CORES = 8
BATCH = 4096
D = 4096          # in/out features (16*16*16)
R = 16            # CP rank
P = 128           # partitions
NB = BATCH // N_CORES   # 512 batch rows per core
BT = NB // P            # 4 batch tiles per core
KT = D // P             # 32 feature k-tiles
KG = 4                  # k-tiles per DMA load group
NG = KT // KG           # 8 load groups
NT = 512                # output column tile (PSUM bank / max moving size)
JT = D // NT            # 8 output column tiles

_PROGRAM = None


def _build_program():
    import concourse.tile as tile
    from concourse import bacc, mybir

    nc = bacc.Bacc(
        "TRN2",
        target_bir_lowering=False,
        debug=False,
        enable_asserts=False,
        num_devices=N_CORES,
    )
    # x^T slice for this core: feature-major, bf16, host-pretransposed and
    # pre-grouped so each k-group load is 128 descriptors x 4 KB contiguous:
    # xg[ng, p, g*NB + b] = x[b, (ng*KG+g)*128 + p]
    xT_d = nc.dram_tensor("xgc", (NG, P, KG * NB), mybir.dt.bfloat16, kind="ExternalInput")
    gin_d = nc.dram_tensor("gin", (P, KT * R), mybir.dt.bfloat16, kind="ExternalInput")
    gout_d = nc.dram_tensor("goutT", (R + 1, D), mybir.dt.bfloat16, kind="ExternalInput")
    # aux: init image of t^T (rows 0..15 zeros, row 16 ones for the bias);
    # DMA-loaded into tT_sb so no on-device memset/matmul is needed
    aux_d = nc.dram_tensor("aux", (R + 1, NB), mybir.dt.bfloat16, kind="ExternalInput")
    y_d = nc.dram_tensor("yc", (NB, D), mybir.dt.float32, kind="ExternalOutput")

    with tile.TileContext(nc) as tc:
        with (
            tc.tile_pool(name="const", bufs=1) as constp,
            tc.tile_pool(name="xT", bufs=NG) as xTp,
            tc.tile_pool(name="tsb", bufs=1) as tsbp,
            tc.tile_pool(name="ysb", bufs=3) as ysbp,
            tc.tile_pool(name="tpsum", bufs=1, space="PSUM") as tpsump,
            tc.tile_pool(name="ypsum", bufs=3, space="PSUM") as ypsump,
        ):
            # const loads on SWDGE: separate queue + semaphore domain from the
            # x loads so stage 1 isn't serialized behind them
            gin_sb = constp.tile([P, KT * R], mybir.dt.bfloat16)
            nc.gpsimd.dma_start(gin_sb[:], gin_d.ap())
            gout_sb = constp.tile([R + 1, D], mybir.dt.bfloat16)
            nc.gpsimd.dma_start(gout_sb[:], gout_d.ap())
            # t^T staging tile, DMA-preloaded with zeros + the bias ones-row
            tT_sb = tsbp.tile([R + 1, NB], mybir.dt.bfloat16)
            nc.gpsimd.dma_start(tT_sb[:], aux_d.ap())

            # x^T loads: 8 groups of 4 k-tiles, each (128, 4*512) bf16 with
            # 4 KB contiguous per partition; spread over all three DMA paths
            xT_sb = []
            for ng in range(NG):
                xt = xTp.tile([P, KG * NB], mybir.dt.bfloat16)
                eng = (nc.sync, nc.scalar, nc.gpsimd)[ng % 3]
                eng.dma_start(xt[:], xT_d.ap()[ng])
                xT_sb.append(xt)

            # stage 1: all 32 k-tiles accumulate into one PSUM tile, but as
            # one start/stop group PER x-load group.  The tile scheduler
            # treats each accumulation group as a unit whose dependencies are
            # the union of its inputs, so one 32-matmul group would stall
            # until the entire x load finished; 8 groups start as their x
            # lands.  PSUM accumulation is per-write on HW, so chaining
            # groups with start=False is exact.
            tT_ps = tpsump.tile([R, NB], mybir.dt.float32)
            for ng in range(NG):
                for g in range(KG):
                    kt = ng * KG + g
                    nc.tensor.matmul(
                        tT_ps[:],
                        lhsT=gin_sb[:, kt * R : (kt + 1) * R],
                        rhs=xT_sb[ng][:, g * NB : (g + 1) * NB],
                        start=(kt == 0),
                        stop=(g == KG - 1),
                        skip_group_check=True,
                    )
            # t^T rows 0..15 = (x@g_in).T, cast bf16 (row 16 = ones via aux)
            nc.vector.tensor_copy(tT_sb[0:R, :], tT_ps[:])

            # stage 2: y = [t,1] @ [g_out.T; bias], per batch tile.
            # jt pairs share a 2-bank PSUM tile; one 1024-col copy per pair,
            # rotated across DVE / ACT / Pool to keep the PE from stalling
            cp = 0
            for bt in range(BT):
                y_sb = ysbp.tile([P, D], mybir.dt.float32)
                for jp in range(JT // 2):
                    y_ps = ypsump.tile([P, 2 * NT], mybir.dt.float32)
                    for h in range(2):
                        jt = jp * 2 + h
                        nc.tensor.matmul(
                            y_ps[:, h * NT : (h + 1) * NT],
                            lhsT=tT_sb[:, bt * P : (bt + 1) * P],
                            rhs=gout_sb[:, jt * NT : (jt + 1) * NT],
                        )
                    dst = y_sb[:, jp * 2 * NT : (jp + 1) * 2 * NT]
                    if cp % 2 == 0:
                        nc.vector.tensor_copy(dst, y_ps[:])
                    else:
                        nc.scalar.copy(dst, y_ps[:])
                    cp += 1
                nc.sync.dma_start(y_d.ap()[bt * P : (bt + 1) * P, :], y_sb[:])

    nc.compile()
    return nc


def _get_program():
    global _PROGRAM
    if _PROGRAM is None:
        _PROGRAM = _build_program()
    return _PROGRAM


def _host_factors(inputs):
    """Build g_in (SBUF layout), [g_out.T; bias], aux (all bf16)."""
    c = [np.asarray(inputs[f"c{i}"], dtype=np.float64) for i in range(6)]
    f = [np.asarray(inputs[f"f{i}"], dtype=np.float64) for i in range(6)]
    bias = np.asarray(inputs["bias"], dtype=np.float32)
    h = [f[i] @ c[i] for i in range(6)]  # (16,16) each
    g_out = (
        h[0][:, None, None, :] * h[1][None, :, None, :] * h[2][None, None, :, :]
    ).reshape(D, R)
    g_in = (
        h[3][:, None, None, :] * h[4][None, :, None, :] * h[5][None, None, :, :]
    ).reshape(D, R)
    # gin SBUF layout: gin_l[p, kt*R + r] = g_in[kt*128 + p, r]
    gin_l = np.ascontiguousarray(
        g_in.reshape(KT, P, R).transpose(1, 0, 2).reshape(P, KT * R)
    ).astype(ml_dtypes.bfloat16)
    goutT = np.concatenate(
        [g_out.T.astype(np.float32), bias[None, :]], axis=0
    ).astype(ml_dtypes.bfloat16)  # (17, 4096)
    aux = np.zeros((R + 1, NB), dtype=ml_dtypes.bfloat16)
    aux[R, :] = 1.0
    return gin_l, goutT, aux


# test-harness hooks (unused in graded path)
TRACE = False
LAST_RESULTS = None


def kernel(**inputs):
    from concourse.bass_utils import run_bass_kernel_spmd

    global LAST_RESULTS
    x = np.asarray(inputs["x"], dtype=np.float32)
    # host-side: cast to bf16, transpose to feature-major, and pre-group:
    # xg[ci][ng, p, g*NB + b] = x[ci*NB + b, (ng*KG+g)*128 + p]
    xb = x.astype(ml_dtypes.bfloat16)  # (BATCH, D)
    xg = np.ascontiguousarray(
        xb.reshape(N_CORES, NB, NG, KG, P).transpose(0, 2, 4, 3, 1)
    ).reshape(N_CORES, NG, P, KG * NB)
    gin_l, goutT, aux = _host_factors(inputs)
    nc = _get_program()
    in_maps = [
        {
            "xgc": xg[ci],
            "gin": gin_l,
            "goutT": goutT,
            "aux": aux,
        }
        for ci in range(N_CORES)
    ]
    res = run_bass_kernel_spmd(
        nc, in_maps, core_ids=list(range(N_CORES)), trace=TRACE
    )
    LAST_RESULTS = res
    y = np.concatenate([r["yc"] for r in res.results], axis=0)
    return np.ascontiguousarray(y.astype(np.float32))


if __name__ == "__main__":
    # quick smoke test with random data
    rng = np.random.default_rng(0)
    ins = {"x": rng.normal(size=(BATCH, D)).astype(np.float32)}
    for i in range(6):
        ins[f"c{i}"] = (rng.normal(size=(8, 16)) * 0.1).astype(np.float32)
        ins[f"f{i}"] = (rng.normal(size=(16, 8)) * 0.1).astype(np.float32)
    ins["bias"] = np.zeros(D, dtype=np.float32)
    y = kernel(**ins)
    print("y", y.shape, y.dtype)


# revision 22
# speedup vs baseline: 1.1858x; 1.1858x over previous
"""Trainium2 Bass kernel for the Kruskal (CP/Tucker) linear layer.

Math: the reference reconstructs W (4096x4096) from a rank-16 CP core and
Tucker factors, then computes y = x @ W.T + bias.  Because the 6D core is a
CP (Kruskal) tensor of rank 16, W itself is exactly rank 16:

    W = g_out @ g_in.T
    g_in[def, r]  = (f3@c3)[d,r] * (f4@c4)[e,r] * (f5@c5)[f,r]   (4096 x 16)
    g_out[abc, r] = (f0@c0)[a,r] * (f1@c1)[b,r] * (f2@c2)[c,r]   (4096 x 16)

so  y = (x @ g_in) @ g_out.T + bias.  The device kernel computes the two
x-dependent projections; the tiny factor-only products (g_in/g_out, ~100
KFLOP) are prepared on the host.

Sharding: data-parallel over the batch (4096 rows -> 8 cores x 512). No
collectives.  The host ships each core its x slice PRE-TRANSPOSED and cast
to bf16 (x^T slice, 4096 features x 512 batch).  Feature-major HBM layout
means stage 1 needs no on-device transpose at all: the contraction dim
lands on partitions straight off the DMA.  Per core:
  1. 8 HWDGE loads of x^T k-groups (128, 4, 512) bf16
  2. stage 1: 32 accumulating matmuls  t^T(16,512) += g_in_kt.T @ x^T_kt
     (N=512, one PSUM bank for the whole core's t^T)
  3. DVE copy t^T -> SBUF bf16 (rank rows + ones row for the bias)
  4. stage 2: 32 bf16 matmuls (4 batch tiles x 8 col tiles, N=512)
     y = [t,1] @ [g_out.T; bias]
  5. DVE/ACT copy PSUM->SBUF, DMA y fp32 out per batch tile
"""

import numpy as np
import ml_dtypes

N_CORES = 8
BATCH = 4096
D = 4096          # in/out features (16*16*16)
R = 16            # CP rank
P = 128           # partitions
NB = BATCH // N_CORES   # 512 batch rows per core
BT = NB // P            # 4 batch tiles per core
KT = D // P             # 32 feature k-tiles
KG = 4                  # k-tiles per DMA load group
NG = KT // KG           # 8 load groups
NT = 512                # output column tile (PSUM bank / max moving size)
JT = D // NT            # 8 output column tiles

_PROGRAM = None


def _build_program():
    import concourse.tile as tile
    from concourse import bacc, mybir

    nc = bacc.Bacc(
        "TRN2",
        target_bir_lowering=False,
        debug=False,
        enable_asserts=False,
        num_devices=N_CORES,
    )
    # x^T slice for this core: feature-major, bf16, host-pretransposed and
    # pre-grouped so each k-group load is 128 descriptors x 4 KB contiguous:
    # xg[ng, p, g*NB + b] = x[b, (ng*KG+g)*128 + p]
    xT_d = nc.dram_tensor("xgc", (NG, P, KG * NB), mybir.dt.bfloat16, kind="ExternalInput")
    gin_d = nc.dram_tensor("gin", (P, KT * R), mybir.dt.bfloat16, kind="ExternalInput")
    gout_d = nc.dram_tensor("goutT", (R + 1, D), mybir.dt.bfloat16, kind="ExternalInput")
    # aux: init image of t^T (rows 0..15 zeros, row 16 ones for the bias);
    # DMA-loaded into tT_sb so no on-device memset/matmul is needed
    aux_d = nc.dram_tensor("aux", (R + 1, NB), mybir.dt.bfloat16, kind="ExternalInput")
    y_d = nc.dram_tensor("yc", (NB, D), mybir.dt.float32, kind="ExternalOutput")

    with tile.TileContext(nc) as tc:
        with (
            tc.tile_pool(name="const", bufs=1) as constp,
            tc.tile_pool(name="xT", bufs=NG) as xTp,
            tc.tile_pool(name="tsb", bufs=1) as tsbp,
            tc.tile_pool(name="ysb", bufs=4) as ysbp,
            tc.tile_pool(name="tpsum", bufs=1, space="PSUM") as tpsump,
            tc.tile_pool(name="ypsum", bufs=3, space="PSUM") as ypsump,
        ):
            # const loads on SWDGE: separate queue + semaphore domain from the
            # x loads so stage 1 isn't serialized behind them
            gin_sb = constp.tile([P, KT * R], mybir.dt.bfloat16)
            nc.gpsimd.dma_start(gin_sb[:], gin_d.ap())
            gout_sb = constp.tile([R + 1, D], mybir.dt.bfloat16)
            nc.gpsimd.dma_start(gout_sb[:], gout_d.ap())
            # t^T staging tile, DMA-preloaded with zeros + the bias ones-row
            tT_sb = tsbp.tile([R + 1, NB], mybir.dt.bfloat16)
            nc.gpsimd.dma_start(tT_sb[:], aux_d.ap())

            # x^T loads: 8 groups of 4 k-tiles, each (128, 4*512) bf16 with
            # 4 KB contiguous per partition; spread over all three DMA paths
            xT_sb = []
            for ng in range(NG):
                xt = xTp.tile([P, KG * NB], mybir.dt.bfloat16)
                eng = (nc.sync, nc.scalar)[ng % 2]
                eng.dma_start(xt[:], xT_d.ap()[ng])
                xT_sb.append(xt)

            # stage 1: all 32 k-tiles accumulate into one PSUM tile, but as
            # one start/stop group PER x-load group.  The tile scheduler
            # treats each accumulation group as a unit whose dependencies are
            # the union of its inputs, so one 32-matmul group would stall
            # until the entire x load finished; 8 groups start as their x
            # lands.  PSUM accumulation is per-write on HW, so chaining
            # groups with start=False is exact.
            tT_ps = tpsump.tile([R, NB], mybir.dt.float32)
            for ng in range(NG):
                for g in range(KG):
                    kt = ng * KG + g
                    nc.tensor.matmul(
                        tT_ps[:],
                        lhsT=gin_sb[:, kt * R : (kt + 1) * R],
                        rhs=xT_sb[ng][:, g * NB : (g + 1) * NB],
                        start=(kt == 0),
                        stop=(g == KG - 1),
                        skip_group_check=True,
                    )
            # t^T rows 0..15 = (x@g_in).T, cast bf16 (row 16 = ones via aux)
            nc.vector.tensor_copy(tT_sb[0:R, :], tT_ps[:])

            # stage 2: y = [t,1] @ [g_out.T; bias], per batch tile.
            # jt pairs share a 2-bank PSUM tile; one 1024-col copy per pair,
            # rotated across DVE / ACT / Pool to keep the PE from stalling
            cp = 0
            for bt in range(BT):
                y_sb = ysbp.tile([P, D], mybir.dt.float32)
                for jp in range(JT // 2):
                    y_ps = ypsump.tile([P, 2 * NT], mybir.dt.float32)
                    for h in range(2):
                        jt = jp * 2 + h
                        nc.tensor.matmul(
                            y_ps[:, h * NT : (h + 1) * NT],
                            lhsT=tT_sb[:, bt * P : (bt + 1) * P],
                            rhs=gout_sb[:, jt * NT : (jt + 1) * NT],
                        )
                    dst = y_sb[:, jp * 2 * NT : (jp + 1) * 2 * NT]
                    if cp % 2 == 0:
                        nc.vector.tensor_copy(dst, y_ps[:])
                    else:
                        nc.scalar.copy(dst, y_ps[:])
                    cp += 1
                yeng = (nc.sync, nc.scalar)[bt % 2]
                yeng.dma_start(y_d.ap()[bt * P : (bt + 1) * P, :], y_sb[:])

    nc.compile()
    return nc


def _get_program():
    global _PROGRAM
    if _PROGRAM is None:
        _PROGRAM = _build_program()
    return _PROGRAM


def _host_factors(inputs):
    """Build g_in (SBUF layout), [g_out.T; bias], aux (all bf16)."""
    c = [np.asarray(inputs[f"c{i}"], dtype=np.float64) for i in range(6)]
    f = [np.asarray(inputs[f"f{i}"], dtype=np.float64) for i in range(6)]
    bias = np.asarray(inputs["bias"], dtype=np.float32)
    h = [f[i] @ c[i] for i in range(6)]  # (16,16) each
    g_out = (
        h[0][:, None, None, :] * h[1][None, :, None, :] * h[2][None, None, :, :]
    ).reshape(D, R)
    g_in = (
        h[3][:, None, None, :] * h[4][None, :, None, :] * h[5][None, None, :, :]
    ).reshape(D, R)
    # gin SBUF layout: gin_l[p, kt*R + r] = g_in[kt*128 + p, r]
    gin_l = np.ascontiguousarray(
        g_in.reshape(KT, P, R).transpose(1, 0, 2).reshape(P, KT * R)
    ).astype(ml_dtypes.bfloat16)
    goutT = np.concatenate(
        [g_out.T.astype(np.float32), bias[None, :]], axis=0
    ).astype(ml_dtypes.bfloat16)  # (17, 4096)
    aux = np.zeros((R + 1, NB), dtype=ml_dtypes.bfloat16)
    aux[R, :] = 1.0
    return gin_l, goutT, aux


# test-harness hooks (unused in graded path)
TRACE = False
LAST_RESULTS = None


def kernel(**inputs):
    from concourse.bass_utils import run_bass_kernel_spmd

    global LAST_RESULTS
    x = np.asarray(inputs["x"], dtype=np.float32)
    # host-side: cast to bf16, transpose to feature-major, and pre-group:
    # xg[ci][ng, p, g*NB + b] = x[ci*NB + b, (ng*KG+g)*128 + p]
    xb = x.astype(ml_dtypes.bfloat16)  # (BATCH, D)
    xg = np.ascontiguousarray(
        xb.reshape(N_CORES, NB, NG, KG, P).transpose(0, 2, 4, 3, 1)
    ).reshape(N_CORES, NG, P, KG * NB)
    gin_l, goutT, aux = _host_factors(inputs)
    nc = _get_program()
    in_maps = [
        {
            "xgc": xg[ci],
            "gin": gin_l,
            "goutT": goutT,
            "aux": aux,
        }
        for ci in range(N_CORES)
    ]
    res = run_bass_kernel_spmd(
        nc, in_maps, core_ids=list(range(N_CORES)), trace=TRACE
    )
    LAST_RESULTS = res
    y = np.concatenate([r["yc"] for r in res.results], axis=0)
    return np.ascontiguousarray(y.astype(np.float32))


if __name__ == "__main__":
    # quick smoke test with random data
    rng = np.random.default_rng(0)
    ins = {"x": rng.normal(size=(BATCH, D)).astype(np.float32)}
    for i in range(6):
        ins[f"c{i}"] = (rng.normal(size=(8, 16)) * 0.1).astype(np.float32)
        ins[f"f{i}"] = (rng.normal(size=(16, 8)) * 0.1).astype(np.float32)
    ins["bias"] = np.zeros(D, dtype=np.float32)
    y = kernel(**ins)
    print("y", y.shape, y.dtype)
